# revision 22
# baseline (speedup 1.0000x reference)
"""Multi-head self-attention (B=4, S=2048, D=1024, H=16) on 8 TRN2 NeuronCores.

Sharding: core c handles batch b = c // 2 and head-group g = c % 2
(8 heads, 512 hidden columns). Per core:
  - Q^T, K^T projections (d-major layout), V projection (token-major),
    all on the tensor engine in fp32r (full-rate fp32).
  - Attention computed transposed: S^T[k, q] = K_h @ Q_h^T per 128-key
    block, exp on the scalar engine (softmax max-subtraction is skipped —
    logits are ~N(0,1), exp cannot overflow), mask applied by accumulating
    a (-8e9 * mask) window into PSUM via an identity matmul.
  - O^T = V_aug^T @ P^T with a ones column appended to V so the softmax
    denominator drops out of the same matmul; normalize by its reciprocal.
  - Row-sharded output projection -> partial [2048, 1024], pairwise
    ReduceScatter between the two cores of a batch, final rows DMA'd out.
Host reassembles: out[b, g*1024:(g+1)*1024, :] = core (2*b + g).
"""

import sys

for _p in ("/opt/trn_rl_repo",):
    if _p not in sys.path:
        sys.path.insert(0, _p)

from contextlib import ExitStack

import ml_dtypes
import numpy as np

import concourse.bass as bass
from concourse import bacc
import concourse.mybir as mybir
import concourse.tile as tile
from concourse.bass_utils import run_bass_kernel_spmd

F32 = mybir.dt.float32
F32R = mybir.dt.float32r
BF16 = mybir.dt.bfloat16
MM_DT = BF16
AF = mybir.ActivationFunctionType
ALU = mybir.AluOpType

B, S, D, H, DEPTH = 4, 2048, 1024, 16, 64
HG = H // 2          # heads per core = 8
GD = HG * DEPTH      # local hidden width = 512
QG = 512             # query-group width (matmul N)
KB = 128             # key-block height (matmul M)
NQG = S // QG        # 4
NKB = S // KB        # 16
NCORES = 8

LAST_EXEC_NS = None
LAST_RESULTS = None


def _mask_schedule(mask2d):
    """Classify each (query-group, key-block) against the actual mask.

    Returns (sched, windows): sched[qg] is a list of (kb, widx|None); a block
    is skipped entirely when fully masked. widx indexes `windows`
    [nwin, 128, 512] holding -8e9 * mask (transposed to [k, q]) for partially
    masked blocks.
    """
    wins = {}
    warr = []
    sched = []
    for qg in range(NQG):
        blocks = []
        for kb in range(NKB):
            blk = mask2d[qg * QG:(qg + 1) * QG, kb * KB:(kb + 1) * KB]  # [q, k]
            if not blk.any():
                blocks.append((kb, None))
            elif (blk == 1.0).all():
                continue
            else:
                assert set(np.unique(blk)) <= {0.0, 1.0}, "non-binary mask"
                w = (1.0 - np.ascontiguousarray(blk.T)).astype(np.float32)
                key = w.tobytes()
                if key not in wins:
                    wins[key] = len(warr)
                    warr.append(w)
                blocks.append((kb, wins[key]))
        sched.append(blocks)
    windows = np.stack(warr) if warr else None
    assert windows is None or len(warr) <= 16, "too many unique mask windows"
    return sched, windows


def _build(sched, windows):
    nc = bacc.Bacc(target_bir_lowering=False, trn_type="TRN2")

    xq = nc.dram_tensor("xq_t", [D, S], BF16, kind="ExternalInput")
    xk = nc.dram_tensor("xk_t", [D, S], BF16, kind="ExternalInput")
    xv = nc.dram_tensor("xv_t", [D, S], BF16, kind="ExternalInput")
    wq_d = nc.dram_tensor("wq_g", [D, GD], BF16, kind="ExternalInput")
    wk_d = nc.dram_tensor("wk_g", [D, GD], BF16, kind="ExternalInput")
    wv_d = nc.dram_tensor("wv_g", [D, GD], BF16, kind="ExternalInput")
    wo_d = nc.dram_tensor("wo_g", [GD, D], BF16, kind="ExternalInput")
    bq_d = nc.dram_tensor("bq_g", [GD], F32, kind="ExternalInput")
    bk_d = nc.dram_tensor("bk_g", [GD], F32, kind="ExternalInput")
    bv_d = nc.dram_tensor("bv_g", [GD], F32, kind="ExternalInput")
    bo_d = nc.dram_tensor("bo_h", [D], F32, kind="ExternalInput")
    out_d = nc.dram_tensor("out", [NQG * (QG // 2), D], F32, kind="ExternalOutput")

    ones_d = nc.inline_tensor(np.ones((128, HG), dtype=ml_dtypes.bfloat16), "ones_col")
    mwin_d = None
    nwin = 0
    if windows is not None:
        nwin = windows.shape[0]
        mwin_d = nc.inline_tensor(windows.astype(ml_dtypes.bfloat16), "mwin")

    with tile.TileContext(nc) as tc, ExitStack() as ctx:
        persist = ctx.enter_context(tc.tile_pool(name="persist", bufs=1))
        dram = ctx.enter_context(tc.tile_pool(name="dram", bufs=1, space="DRAM"))

        partials = [dram.tile([QG, D], F32, tag=f"partial{qg}", name=f"partial{qg}")
                    for qg in range(NQG)]
        rs_outs = [dram.tile([QG // 4, D], F32, tag=f"rs{ch}", name=f"rs{ch}")
                   for ch in range(2 * NQG)]

        # ---- persistent SBUF tensors -------------------------------------
        qt = [persist.tile([128, S], MM_DT, tag=f"qt{i}", name=f"qt{i}") for i in range(4)]
        kt = [persist.tile([128, S], MM_DT, tag=f"kt{i}", name=f"kt{i}") for i in range(4)]
        # V with a ones column appended per head: [tokens, head, 64 V + 1]
        vt = [persist.tile([128, HG, DEPTH + 1], MM_DT, tag=f"vt{t}", name=f"vt{t}")
              for t in range(NKB)]
        wosb = persist.tile([128, 4, D], MM_DT, tag="wosb")
        mw = [persist.tile([KB, QG], MM_DT, tag=f"mw{w}", name=f"mw{w}") for w in range(nwin)]
        bo_bc = persist.tile([128, D], F32, tag="bo_bc")
        bv_bc = persist.tile([128, GD], F32, tag="bv_bc")
        bqt = [persist.tile([128, 1], F32, tag=f"bq{m}", name=f"bq{m}") for m in range(4)]
        bkt = [persist.tile([128, 1], F32, tag=f"bk{m}", name=f"bk{m}") for m in range(4)]

        for m in range(4):
            nc.gpsimd.dma_start(out=bqt[m], in_=bq_d[m * 128:(m + 1) * 128])
            nc.gpsimd.dma_start(out=bkt[m], in_=bk_d[m * 128:(m + 1) * 128])

        def load_constants():
            for w in range(nwin):
                nc.gpsimd.dma_start(out=mw[w], in_=mwin_d[w, :, :])
            nc.gpsimd.dma_start(
                out=bo_bc,
                in_=bass.AP(tensor=bo_d, offset=0, ap=[[0, 128], [1, D]]),
            )
            nc.gpsimd.dma_start(
                out=bv_bc,
                in_=bass.AP(tensor=bv_d, offset=0, ap=[[0, 128], [1, GD]]),
            )
            for t in range(NKB):
                nc.gpsimd.dma_start(
                    out=vt[t][:, :, DEPTH:DEPTH + 1], in_=ones_d[:, :]
                )

        pps = ctx.enter_context(tc.tile_pool(name="pps", bufs=2, space="PSUM"))
        pacc = ctx.enter_context(tc.tile_pool(name="pacc", bufs=2, space="PSUM"))

        # ---- phase 1: projections ---------------------------------------
        with tc.tile_pool(name="xw", bufs=2) as xw_pool, \
             tc.tile_pool(name="wp", bufs=2) as w_pool:

            def proj_T(xd, wd, bias_tiles, dst):
                """dst[m][:, tg*512...] = (x @ w + b)^T  (d on partitions)."""
                wt = w_pool.tile([128, 8, GD], MM_DT, tag="wt")
                for kk in range(8):
                    eng = (nc.scalar, nc.sync, nc.gpsimd)[kk % 3]
                    eng.dma_start(
                        out=wt[:, kk, :], in_=wd[kk * 128:(kk + 1) * 128, :]
                    )
                xt = xw_pool.tile([128, 8, S], MM_DT, tag="xt")
                for kk in range(8):
                    for hf in range(2):
                        eng = (nc.sync, nc.scalar, nc.gpsimd)[(2 * kk + hf) % 3]
                        eng.dma_start(
                            out=xt[:, kk, hf * (S // 2):(hf + 1) * (S // 2)],
                            in_=xd[kk * 128:(kk + 1) * 128,
                                   hf * (S // 2):(hf + 1) * (S // 2)],
                        )
                for tg in range(4):
                    for mh in range(2):
                        ps = pps.tile([128, 2 * QG], F32, tag="ps")
                        for mm_ in range(2):
                            m = mh * 2 + mm_
                            for kk in range(8):
                                nc.tensor.matmul(
                                    ps[:, mm_ * QG:(mm_ + 1) * QG],
                                    wt[:, kk, m * 128:(m + 1) * 128],
                                    xt[:, kk, tg * QG:(tg + 1) * QG],
                                    start=(kk == 0),
                                    stop=(kk == 7),
                                )
                        for mm_ in range(2):
                            m = mh * 2 + mm_
                            nc.vector.tensor_scalar_add(
                                dst[m][:, tg * QG:(tg + 1) * QG],
                                ps[:, mm_ * QG:(mm_ + 1) * QG],
                                bias_tiles[m][:, :],
                            )

            proj_T(xq, wq_d, bqt, qt)
            load_constants()
            proj_T(xk, wk_d, bkt, kt)

            # V projection: token-major, bias added on eviction
            wt = w_pool.tile([128, 8, GD], MM_DT, tag="wt")
            for kk in range(8):
                eng = (nc.scalar, nc.sync, nc.gpsimd)[kk % 3]
                eng.dma_start(
                    out=wt[:, kk, :], in_=wv_d[kk * 128:(kk + 1) * 128, :]
                )
            xt = xw_pool.tile([128, 8, S], MM_DT, tag="xt")
            for kk in range(8):
                for hf in range(2):
                    eng = (nc.sync, nc.scalar, nc.gpsimd)[(2 * kk + hf) % 3]
                    eng.dma_start(
                        out=xt[:, kk, hf * (S // 2):(hf + 1) * (S // 2)],
                        in_=xv[kk * 128:(kk + 1) * 128,
                               hf * (S // 2):(hf + 1) * (S // 2)],
                    )
            for tg in range(4):
                for th in range(2):
                    ps = pps.tile([128, 2 * GD], F32, tag="ps")
                    for ts_ in range(2):
                        ts = th * 2 + ts_
                        for kk in range(8):
                            nc.tensor.matmul(
                                ps[:, ts_ * GD:(ts_ + 1) * GD],
                                xt[:, kk, tg * QG + ts * 128: tg * QG + (ts + 1) * 128],
                                wt[:, kk, :],
                                start=(kk == 0),
                                stop=(kk == 7),
                            )
                    for ts_ in range(2):
                        t = tg * 4 + th * 2 + ts_
                        nc.vector.tensor_tensor(
                            vt[t][:, :, 0:DEPTH],
                            ps[:, ts_ * GD:(ts_ + 1) * GD].rearrange(
                                "p (h d) -> p h d", h=HG),
                            bv_bc[:, :].rearrange("p (h d) -> p h d", h=HG),
                            ALU.add,
                        )

            # wo: [512, 1024] -> 4 contraction tiles
            for kk in range(4):
                nc.sync.dma_start(
                    out=wosb[:, kk, :], in_=wo_d[kk * 128:(kk + 1) * 128, :]
                )

        # ---- phase 2: attention + output projection ---------------------
        with tc.tile_pool(name="ptp", bufs=20) as pt_pool, \
             tc.tile_pool(name="otp", bufs=2) as ot_pool, \
             tc.tile_pool(name="nrm", bufs=2) as nrm_pool, \
             tc.tile_pool(name="osb", bufs=3) as osb_pool:

            for qg in (1, 2, 3, 0):
                blocks = sched[qg]
                nb = len(blocks)
                qgc = slice(qg * QG, (qg + 1) * QG)
                ot = [ot_pool.tile([128, QG], MM_DT, tag=f"ot{i}", name=f"ot{i}") for i in range(4)]
                for i in range(4):  # head pair: heads 2i (rows 0:64), 2i+1 (64:128)
                    # scores stretch: both heads of the pair share one
                    # [128, 1024] PSUM tile (2 banks) and one exp pass
                    pts = []
                    for kb, wix in blocks:
                        kbc = slice(kb * KB, (kb + 1) * KB)
                        sps = pps.tile([128, 2 * QG], F32, tag="ps")
                        for p, off in ((0, 0), (1, 64)):
                            nc.tensor.matmul(
                                sps[:, p * QG:(p + 1) * QG],
                                kt[i][off:off + 64, kbc],
                                qt[i][off:off + 64, qgc],
                                start=True,
                                stop=True,
                            )
                        pt = pt_pool.tile([KB, 2 * QG], MM_DT, tag="pt")
                        nc.scalar.activation(
                            pt[:, :], sps[:, :], AF.Exp, scale=0.125
                        )
                        if wix is not None:
                            m_ap = mw[wix][:, :]
                            rep = bass.AP(
                                tensor=m_ap.tensor,
                                offset=m_ap.offset,
                                ap=[list(m_ap.ap[0]), [0, 2], list(m_ap.ap[1])],
                            )
                            nc.vector.tensor_tensor(
                                pt[:, :].rearrange("k (t q) -> k t q", t=2),
                                pt[:, :].rearrange("k (t q) -> k t q", t=2),
                                rep,
                                ALU.mult,
                            )
                        pts.append((kb, pt))
                    # AV stretch: accumulate both heads (128x128 mode throughout)
                    accs = [pacc.tile([128, QG], F32, tag=f"acc{p}", name=f"acc{p}")
                            for p in range(2)]
                    for bi, (kb, pt) in enumerate(pts):
                        for p in range(2):
                            nc.tensor.matmul(
                                accs[p][0:DEPTH + 1, :],
                                vt[kb][:, 2 * i + p, :],
                                pt[:, p * QG:(p + 1) * QG],
                                start=(bi == 0),
                                stop=(bi == nb - 1),
                            )
                    # evict AV result to SBUF (frees the PSUM bank fast),
                    # then normalize: O^T = o_un[0:64] * (1 / o_un[64])
                    for p, acc in enumerate(accs):
                        o_un = nrm_pool.tile([DEPTH + 1, QG], F32, tag="o_un")
                        nc.vector.tensor_copy(o_un[:, :], acc[0:DEPTH + 1, :])
                        rc0 = nrm_pool.tile([1, QG], F32, tag="rc0")
                        nc.scalar.dma_start(out=rc0[:, :], in_=o_un[64:65, :])
                        rcr = nrm_pool.tile([1, QG], F32, tag="rcr")
                        nc.vector.reciprocal_approx_fast(rcr[:, :], rc0[:, :])
                        rb = nrm_pool.tile([64, QG], F32, tag="rb")
                        nc.gpsimd.partition_broadcast(rb[:, :], rcr[:, :])
                        if p == 0:
                            nc.vector.tensor_tensor(
                                ot[i][0:64, :], o_un[0:64, :], rb[:, :], ALU.mult
                            )
                        else:
                            tmp = nrm_pool.tile([64, QG], MM_DT, tag="tmp")
                            nc.vector.tensor_tensor(
                                tmp[:, :], o_un[0:64, :], rb[:, :], ALU.mult
                            )
                            nc.scalar.dma_start(out=ot[i][64:128, :], in_=tmp[:, :])

                # output projection for this query group
                for ts in range(4):
                    po = pps.tile([128, D], F32, tag="ps")
                    for nn in range(2):
                        for kk in range(4):
                            nc.tensor.matmul(
                                po[:, nn * QG:(nn + 1) * QG],
                                ot[kk][:, ts * 128:(ts + 1) * 128],
                                wosb[:, kk, nn * QG:(nn + 1) * QG],
                                start=(kk == 0),
                                stop=(kk == 3),
                            )
                    osb = osb_pool.tile([128, D], F32, tag="osb")
                    nc.vector.tensor_tensor(
                        osb[:, :], po[:, :], bo_bc[:, :], ALU.add,
                    )
                    nc.sync.dma_start(
                        out=partials[qg][ts * 128:(ts + 1) * 128, :],
                        in_=osb[:, :],
                    )
                    if ts % 2 == 1:
                        # reduce-scatter the finished 256-row half with the peer
                        hh = ts // 2
                        ch = 2 * qg + hh
                        nc.gpsimd.collective_compute(
                            "ReduceScatter",
                            ALU.add,
                            replica_groups=[[0, 1], [2, 3], [4, 5], [6, 7]],
                            ins=[partials[qg][hh * 256:(hh + 1) * 256, :]],
                            outs=[rs_outs[ch][:, :]],
                        )
                        nc.sync.dma_start(
                            out=out_d[ch * 128:(ch + 1) * 128, :],
                            in_=rs_outs[ch][:, :],
                        )

    nc.finalize()
    return nc


_CACHED = {}


def _get_nc(mask2d):
    key = mask2d.tobytes()
    if key not in _CACHED:
        _CACHED[key] = _build(*_mask_schedule(mask2d))
    return _CACHED[key]


def kernel(v, k, q, mask, wq, bq, wk, bk, wv, bv, wo, bo, _trace=False):
    global LAST_EXEC_NS, LAST_RESULTS
    f = lambda a: np.asarray(a, dtype=np.float32)
    v, k, q = f(v), f(k), f(q)
    wq, wk, wv, wo = f(wq), f(wk), f(wv), f(wo)
    bq, bk, bv, bo = f(bq), f(bk), f(bv), f(bo)
    mask2d = f(mask).reshape(S, S)

    nc = _get_nc(mask2d)

    bf = lambda a: np.ascontiguousarray(a).astype(ml_dtypes.bfloat16)
    in_maps = []
    for c in range(NCORES):
        b, g = c // 2, c % 2
        cols = slice(g * GD, (g + 1) * GD)
        in_maps.append({
            "xq_t": bf(q[b].T),
            "xk_t": bf(k[b].T),
            "xv_t": bf(v[b].T),
            "wq_g": bf(wq[:, cols]),
            "wk_g": bf(wk[:, cols]),
            "wv_g": bf(wv[:, cols]),
            "wo_g": bf(wo[cols, :]),
            "bq_g": np.ascontiguousarray(bq[cols]),
            "bk_g": np.ascontiguousarray(bk[cols]),
            "bv_g": np.ascontiguousarray(bv[cols]),
            "bo_h": np.ascontiguousarray(bo * np.float32(0.5)),
        })

    res = run_bass_kernel_spmd(
        nc, in_maps, core_ids=list(range(NCORES)), trace=_trace
    )
    LAST_EXEC_NS = res.exec_time_ns
    LAST_RESULTS = res

    out = np.empty((B, S, D), dtype=np.float32)
    for c in range(NCORES):
        b, g = c // 2, c % 2
        o = res.results[c]["out"]  # [8 * 128, D]; chunk ch covers source rows
        for ch in range(8):
            qg, hh = ch // 2, ch % 2
            r0 = qg * QG + hh * 256 + g * 128
            out[b, r0:r0 + 128, :] = o[ch * 128:(ch + 1) * 128, :]
    return out


# revision 23
# speedup vs baseline: 1.0610x; 1.0610x over previous
"""Multi-head self-attention (B=4, S=2048, D=1024, H=16) on 8 TRN2 NeuronCores.

Sharding: core c handles batch b = c // 2 and head-group g = c % 2
(8 heads, 512 hidden columns). Per core:
  - Q^T, K^T projections (d-major layout), V projection (token-major),
    all on the tensor engine in fp32r (full-rate fp32).
  - Attention computed transposed: S^T[k, q] = K_h @ Q_h^T per 128-key
    block, exp on the scalar engine (softmax max-subtraction is skipped —
    logits are ~N(0,1), exp cannot overflow), mask applied by accumulating
    a (-8e9 * mask) window into PSUM via an identity matmul.
  - O^T = V_aug^T @ P^T with a ones column appended to V so the softmax
    denominator drops out of the same matmul; normalize by its reciprocal.
  - Row-sharded output projection -> partial [2048, 1024], pairwise
    ReduceScatter between the two cores of a batch, final rows DMA'd out.
Host reassembles: out[b, g*1024:(g+1)*1024, :] = core (2*b + g).
"""

import sys

for _p in ("/opt/trn_rl_repo",):
    if _p not in sys.path:
        sys.path.insert(0, _p)

from contextlib import ExitStack

import ml_dtypes
import numpy as np

import concourse.bass as bass
from concourse import bacc
import concourse.mybir as mybir
import concourse.tile as tile
from concourse.bass_utils import run_bass_kernel_spmd

F32 = mybir.dt.float32
F32R = mybir.dt.float32r
BF16 = mybir.dt.bfloat16
MM_DT = BF16
AF = mybir.ActivationFunctionType
ALU = mybir.AluOpType

B, S, D, H, DEPTH = 4, 2048, 1024, 16, 64
HG = H // 2          # heads per core = 8
GD = HG * DEPTH      # local hidden width = 512
QG = 512             # query-group width (matmul N)
KB = 128             # key-block height (matmul M)
NQG = S // QG        # 4
NKB = S // KB        # 16
NCORES = 8

LAST_EXEC_NS = None
LAST_RESULTS = None

QG_ORDER = (1, 2, 3, 0)


def _rs_chunks():
    """(qg, r0, nrows, out_row0) per reduce-scatter chunk; finer chunks for
    the last query group processed so the exposed tail is small."""
    chunks = []
    for qg in QG_ORDER:
        step = 128 if qg == QG_ORDER[-1] else 256
        for r0 in range(0, QG, step):
            chunks.append((qg, r0, step, qg * (QG // 2) + r0 // 2))
    return chunks


def _mask_schedule(mask2d):
    """Classify each (query-group, key-block) against the actual mask.

    Returns (sched, windows): sched[qg] is a list of (kb, widx|None); a block
    is skipped entirely when fully masked. widx indexes `windows`
    [nwin, 128, 512] holding -8e9 * mask (transposed to [k, q]) for partially
    masked blocks.
    """
    wins = {}
    warr = []
    sched = []
    for qg in range(NQG):
        blocks = []
        for kb in range(NKB):
            blk = mask2d[qg * QG:(qg + 1) * QG, kb * KB:(kb + 1) * KB]  # [q, k]
            if not blk.any():
                blocks.append((kb, None))
            elif (blk == 1.0).all():
                continue
            else:
                assert set(np.unique(blk)) <= {0.0, 1.0}, "non-binary mask"
                w = (1.0 - np.ascontiguousarray(blk.T)).astype(np.float32)
                key = w.tobytes()
                if key not in wins:
                    wins[key] = len(warr)
                    warr.append(w)
                blocks.append((kb, wins[key]))
        sched.append(blocks)
    windows = np.stack(warr) if warr else None
    assert windows is None or len(warr) <= 16, "too many unique mask windows"
    return sched, windows


def _build(sched, windows):
    nc = bacc.Bacc(target_bir_lowering=False, trn_type="TRN2")

    xq = nc.dram_tensor("xq_t", [D, S], BF16, kind="ExternalInput")
    xk = nc.dram_tensor("xk_t", [D, S], BF16, kind="ExternalInput")
    xv = nc.dram_tensor("xv_t", [D, S], BF16, kind="ExternalInput")
    wq_d = nc.dram_tensor("wq_g", [D, GD], BF16, kind="ExternalInput")
    wk_d = nc.dram_tensor("wk_g", [D, GD], BF16, kind="ExternalInput")
    wv_d = nc.dram_tensor("wv_g", [D, GD], BF16, kind="ExternalInput")
    wo_d = nc.dram_tensor("wo_g", [GD, D], BF16, kind="ExternalInput")
    bq_d = nc.dram_tensor("bq_g", [GD], F32, kind="ExternalInput")
    bk_d = nc.dram_tensor("bk_g", [GD], F32, kind="ExternalInput")
    bv_d = nc.dram_tensor("bv_g", [GD], F32, kind="ExternalInput")
    bo_d = nc.dram_tensor("bo_h", [D], F32, kind="ExternalInput")
    out_d = nc.dram_tensor("out", [NQG * (QG // 2), D], F32, kind="ExternalOutput")

    ones_d = nc.inline_tensor(np.ones((128, HG), dtype=ml_dtypes.bfloat16), "ones_col")
    mwin_d = None
    nwin = 0
    if windows is not None:
        nwin = windows.shape[0]
        mwin_d = nc.inline_tensor(windows.astype(ml_dtypes.bfloat16), "mwin")

    with tile.TileContext(nc) as tc, ExitStack() as ctx:
        persist = ctx.enter_context(tc.tile_pool(name="persist", bufs=1))
        dram = ctx.enter_context(tc.tile_pool(name="dram", bufs=1, space="DRAM"))

        partials = [dram.tile([QG, D], F32, tag=f"partial{qg}", name=f"partial{qg}")
                    for qg in range(NQG)]
        chunks = _rs_chunks()
        rs_outs = [dram.tile([nrows // 2, D], F32, tag=f"rs{ci}", name=f"rs{ci}")
                   for ci, (cqg, r0, nrows, orow) in enumerate(chunks)]

        # ---- persistent SBUF tensors -------------------------------------
        qt = [persist.tile([128, S], MM_DT, tag=f"qt{i}", name=f"qt{i}") for i in range(4)]
        kt = [persist.tile([128, S], MM_DT, tag=f"kt{i}", name=f"kt{i}") for i in range(4)]
        # V with a ones column appended per head: [tokens, head, 64 V + 1]
        vt = [persist.tile([128, HG, DEPTH + 1], MM_DT, tag=f"vt{t}", name=f"vt{t}")
              for t in range(NKB)]
        wosb = persist.tile([128, 4, D], MM_DT, tag="wosb")
        mw = [persist.tile([KB, QG], MM_DT, tag=f"mw{w}", name=f"mw{w}") for w in range(nwin)]
        bo_bc = persist.tile([128, D], F32, tag="bo_bc")
        bv_bc = persist.tile([128, GD], F32, tag="bv_bc")
        bqt = [persist.tile([128, 1], F32, tag=f"bq{m}", name=f"bq{m}") for m in range(4)]
        bkt = [persist.tile([128, 1], F32, tag=f"bk{m}", name=f"bk{m}") for m in range(4)]

        for m in range(4):
            nc.gpsimd.dma_start(out=bqt[m], in_=bq_d[m * 128:(m + 1) * 128])
            nc.gpsimd.dma_start(out=bkt[m], in_=bk_d[m * 128:(m + 1) * 128])

        def load_constants():
            for w in range(nwin):
                nc.gpsimd.dma_start(out=mw[w], in_=mwin_d[w, :, :])
            nc.gpsimd.dma_start(
                out=bo_bc,
                in_=bass.AP(tensor=bo_d, offset=0, ap=[[0, 128], [1, D]]),
            )
            nc.gpsimd.dma_start(
                out=bv_bc,
                in_=bass.AP(tensor=bv_d, offset=0, ap=[[0, 128], [1, GD]]),
            )
            for t in range(NKB):
                nc.gpsimd.dma_start(
                    out=vt[t][:, :, DEPTH:DEPTH + 1], in_=ones_d[:, :]
                )

        pps = ctx.enter_context(tc.tile_pool(name="pps", bufs=2, space="PSUM"))
        ppo = ctx.enter_context(tc.tile_pool(name="ppo", bufs=1, space="PSUM"))
        pacc = ctx.enter_context(tc.tile_pool(name="pacc", bufs=3, space="PSUM"))

        # ---- phase 1: projections ---------------------------------------
        with tc.tile_pool(name="xw", bufs=2) as xw_pool, \
             tc.tile_pool(name="wp", bufs=2) as w_pool:

            def proj_T(xd, wd, bias_tiles, dst):
                """dst[m][:, tg*512...] = (x @ w + b)^T  (d on partitions)."""
                wt = w_pool.tile([128, 8, GD], MM_DT, tag="wt")
                for kk in range(8):
                    eng = (nc.scalar, nc.sync, nc.gpsimd)[kk % 3]
                    eng.dma_start(
                        out=wt[:, kk, :], in_=wd[kk * 128:(kk + 1) * 128, :]
                    )
                xt = xw_pool.tile([128, 8, S], MM_DT, tag="xt")
                for kk in range(8):
                    for hf in range(2):
                        eng = (nc.sync, nc.scalar, nc.gpsimd)[(2 * kk + hf) % 3]
                        eng.dma_start(
                            out=xt[:, kk, hf * (S // 2):(hf + 1) * (S // 2)],
                            in_=xd[kk * 128:(kk + 1) * 128,
                                   hf * (S // 2):(hf + 1) * (S // 2)],
                        )
                for tg in range(4):
                    for mh in range(2):
                        ps = pps.tile([128, 2 * QG], F32, tag="ps")
                        for mm_ in range(2):
                            m = mh * 2 + mm_
                            for kk in range(8):
                                nc.tensor.matmul(
                                    ps[:, mm_ * QG:(mm_ + 1) * QG],
                                    wt[:, kk, m * 128:(m + 1) * 128],
                                    xt[:, kk, tg * QG:(tg + 1) * QG],
                                    start=(kk == 0),
                                    stop=(kk == 7),
                                )
                        for mm_ in range(2):
                            m = mh * 2 + mm_
                            nc.vector.tensor_scalar_add(
                                dst[m][:, tg * QG:(tg + 1) * QG],
                                ps[:, mm_ * QG:(mm_ + 1) * QG],
                                bias_tiles[m][:, :],
                            )

            proj_T(xq, wq_d, bqt, qt)
            proj_T(xk, wk_d, bkt, kt)
            load_constants()

            # V projection: token-major, bias added on eviction
            wt = w_pool.tile([128, 8, GD], MM_DT, tag="wt")
            for kk in range(8):
                eng = (nc.scalar, nc.sync, nc.gpsimd)[kk % 3]
                eng.dma_start(
                    out=wt[:, kk, :], in_=wv_d[kk * 128:(kk + 1) * 128, :]
                )
            xt = xw_pool.tile([128, 8, S], MM_DT, tag="xt")
            for kk in range(8):
                for hf in range(2):
                    eng = (nc.sync, nc.scalar, nc.gpsimd)[(2 * kk + hf) % 3]
                    eng.dma_start(
                        out=xt[:, kk, hf * (S // 2):(hf + 1) * (S // 2)],
                        in_=xv[kk * 128:(kk + 1) * 128,
                               hf * (S // 2):(hf + 1) * (S // 2)],
                    )
            for tg in range(4):
                for th in range(2):
                    ps = pps.tile([128, 2 * GD], F32, tag="ps")
                    for ts_ in range(2):
                        ts = th * 2 + ts_
                        for kk in range(8):
                            nc.tensor.matmul(
                                ps[:, ts_ * GD:(ts_ + 1) * GD],
                                xt[:, kk, tg * QG + ts * 128: tg * QG + (ts + 1) * 128],
                                wt[:, kk, :],
                                start=(kk == 0),
                                stop=(kk == 7),
                            )
                    for ts_ in range(2):
                        t = tg * 4 + th * 2 + ts_
                        nc.vector.tensor_tensor(
                            vt[t][:, :, 0:DEPTH],
                            ps[:, ts_ * GD:(ts_ + 1) * GD].rearrange(
                                "p (h d) -> p h d", h=HG),
                            bv_bc[:, :].rearrange("p (h d) -> p h d", h=HG),
                            ALU.add,
                        )

            # wo: [512, 1024] -> 4 contraction tiles
            for kk in range(4):
                nc.sync.dma_start(
                    out=wosb[:, kk, :], in_=wo_d[kk * 128:(kk + 1) * 128, :]
                )

        # ---- phase 2: attention + output projection ---------------------
        with tc.tile_pool(name="ptp", bufs=20) as pt_pool, \
             tc.tile_pool(name="otp", bufs=2) as ot_pool, \
             tc.tile_pool(name="nrm", bufs=2) as nrm_pool, \
             tc.tile_pool(name="osb", bufs=3) as osb_pool:

            for qg in QG_ORDER:
                blocks = sched[qg]
                nb = len(blocks)
                qgc = slice(qg * QG, (qg + 1) * QG)
                ot = [ot_pool.tile([128, QG], MM_DT, tag=f"ot{i}", name=f"ot{i}") for i in range(4)]
                for i in range(4):  # head pair: heads 2i (rows 0:64), 2i+1 (64:128)
                    # scores stretch: both heads of the pair share one
                    # [128, 1024] PSUM tile (2 banks) and one exp pass
                    pts = []
                    for kb, wix in blocks:
                        kbc = slice(kb * KB, (kb + 1) * KB)
                        sps = pps.tile([128, 2 * QG], F32, tag="ps")
                        for p, off in ((0, 0), (1, 64)):
                            nc.tensor.matmul(
                                sps[:, p * QG:(p + 1) * QG],
                                kt[i][off:off + 64, kbc],
                                qt[i][off:off + 64, qgc],
                                start=True,
                                stop=True,
                            )
                        pt = pt_pool.tile([KB, 2 * QG], MM_DT, tag="pt")
                        nc.scalar.activation(
                            pt[:, :], sps[:, :], AF.Exp, scale=0.125
                        )
                        if wix is not None:
                            m_ap = mw[wix][:, :]
                            rep = bass.AP(
                                tensor=m_ap.tensor,
                                offset=m_ap.offset,
                                ap=[list(m_ap.ap[0]), [0, 2], list(m_ap.ap[1])],
                            )
                            nc.vector.tensor_tensor(
                                pt[:, :].rearrange("k (t q) -> k t q", t=2),
                                pt[:, :].rearrange("k (t q) -> k t q", t=2),
                                rep,
                                ALU.mult,
                            )
                        pts.append((kb, pt))
                    # AV stretch: accumulate both heads (128x128 mode throughout)
                    accs = [pacc.tile([128, QG], F32, tag="acc", name=f"acc{p}")
                            for p in range(2)]
                    for bi, (kb, pt) in enumerate(pts):
                        for p in range(2):
                            nc.tensor.matmul(
                                accs[p][0:DEPTH + 1, :],
                                vt[kb][:, 2 * i + p, :],
                                pt[:, p * QG:(p + 1) * QG],
                                start=(bi == 0),
                                stop=(bi == nb - 1),
                            )
                    # evict AV result to SBUF (frees the PSUM bank fast),
                    # then normalize: O^T = o_un[0:64] * (1 / o_un[64])
                    for p, acc in enumerate(accs):
                        o_un = nrm_pool.tile([DEPTH + 1, QG], F32, tag="o_un")
                        nc.vector.tensor_copy(o_un[:, :], acc[0:DEPTH + 1, :])
                        rc0 = nrm_pool.tile([1, QG], F32, tag="rc0")
                        nc.scalar.dma_start(out=rc0[:, :], in_=o_un[64:65, :])
                        rcr = nrm_pool.tile([1, QG], F32, tag="rcr")
                        nc.vector.reciprocal_approx_fast(rcr[:, :], rc0[:, :])
                        rb = nrm_pool.tile([64, QG], F32, tag="rb")
                        nc.gpsimd.partition_broadcast(rb[:, :], rcr[:, :])
                        if p == 0:
                            nc.vector.tensor_tensor(
                                ot[i][0:64, :], o_un[0:64, :], rb[:, :], ALU.mult
                            )
                        else:
                            tmp = nrm_pool.tile([64, QG], MM_DT, tag="tmp")
                            nc.vector.tensor_tensor(
                                tmp[:, :], o_un[0:64, :], rb[:, :], ALU.mult
                            )
                            nc.scalar.dma_start(out=ot[i][64:128, :], in_=tmp[:, :])

                # output projection for this query group
                for ts in range(4):
                    for nn in range(2):
                        po = ppo.tile([128, QG], F32, tag="po")
                        for kk in range(4):
                            nc.tensor.matmul(
                                po[:, :],
                                ot[kk][:, ts * 128:(ts + 1) * 128],
                                wosb[:, kk, nn * QG:(nn + 1) * QG],
                                start=(kk == 0),
                                stop=(kk == 3),
                            )
                        osb = osb_pool.tile([128, QG], F32, tag="osb")
                        nc.vector.tensor_tensor(
                            osb[:, :], po[:, :],
                            bo_bc[:, nn * QG:(nn + 1) * QG], ALU.add,
                        )
                        nc.sync.dma_start(
                            out=partials[qg][ts * 128:(ts + 1) * 128,
                                             nn * QG:(nn + 1) * QG],
                            in_=osb[:, :],
                        )
                    # fire any reduce-scatter chunk that just completed
                    for ci, (cqg, r0, nrows, orow) in enumerate(chunks):
                        if cqg == qg and r0 + nrows == (ts + 1) * 128:
                            nc.gpsimd.collective_compute(
                                "ReduceScatter",
                                ALU.add,
                                replica_groups=[[0, 1], [2, 3], [4, 5], [6, 7]],
                                ins=[partials[qg][r0:r0 + nrows, :]],
                                outs=[rs_outs[ci][:, :]],
                            )
                            nc.sync.dma_start(
                                out=out_d[orow:orow + nrows // 2, :],
                                in_=rs_outs[ci][:, :],
                            )

    nc.finalize()
    return nc


_CACHED = {}


def _get_nc(mask2d):
    key = mask2d.tobytes()
    if key not in _CACHED:
        _CACHED[key] = _build(*_mask_schedule(mask2d))
    return _CACHED[key]


def kernel(v, k, q, mask, wq, bq, wk, bk, wv, bv, wo, bo, _trace=False):
    global LAST_EXEC_NS, LAST_RESULTS
    f = lambda a: np.asarray(a, dtype=np.float32)
    v, k, q = f(v), f(k), f(q)
    wq, wk, wv, wo = f(wq), f(wk), f(wv), f(wo)
    bq, bk, bv, bo = f(bq), f(bk), f(bv), f(bo)
    mask2d = f(mask).reshape(S, S)

    nc = _get_nc(mask2d)

    bf = lambda a: np.ascontiguousarray(a).astype(ml_dtypes.bfloat16)
    in_maps = []
    for c in range(NCORES):
        b, g = c // 2, c % 2
        cols = slice(g * GD, (g + 1) * GD)
        in_maps.append({
            "xq_t": bf(q[b].T),
            "xk_t": bf(k[b].T),
            "xv_t": bf(v[b].T),
            "wq_g": bf(wq[:, cols]),
            "wk_g": bf(wk[:, cols]),
            "wv_g": bf(wv[:, cols]),
            "wo_g": bf(wo[cols, :]),
            "bq_g": np.ascontiguousarray(bq[cols]),
            "bk_g": np.ascontiguousarray(bk[cols]),
            "bv_g": np.ascontiguousarray(bv[cols]),
            "bo_h": np.ascontiguousarray(bo * np.float32(0.5)),
        })

    res = run_bass_kernel_spmd(
        nc, in_maps, core_ids=list(range(NCORES)), trace=_trace
    )
    LAST_EXEC_NS = res.exec_time_ns
    LAST_RESULTS = res

    out = np.empty((B, S, D), dtype=np.float32)
    chunks = _rs_chunks()
    for c in range(NCORES):
        b, g = c // 2, c % 2
        o = res.results[c]["out"]  # [1024, D]
        for (qg, r0, nrows, orow) in chunks:
            gr = qg * QG + r0 + g * (nrows // 2)
            out[b, gr:gr + nrows // 2, :] = o[orow:orow + nrows // 2, :]
    return out
